# revision 1
# baseline (speedup 1.0000x reference)
"""Trainium2 Bass kernel for nn_ExactDivergenceModel (retrieval_knn).

Backend reality (measured): each run of a program through this axon stack
costs ~30-50us PER STATIC NEFF INSTRUCTION in dispatch overhead, while
hardware-loop iterations are nearly free (just engine compute). The
straightforward kernel (~690 static instructions) therefore costs ~20ms;
this one restructures the identical math into per-engine hardware Fori
loops over the 32 row-blocks -> ~55 static instructions.

Math per batch b (one NeuronCore each):
  v_xx[i,j] = x2[j] - 2<X_i,X_j>,  v_yx[i,j] = y2[j] - 2<X_i,Y_j>
  d2min[i]  = min_j v[i,j] + x2[i]        (XX: diag masked with +2^30)
  out[b]    = 0.5 * mean_i(log d2min_xx[i] - log d2min_yx[i])

Per round q = bi (HW loop on every engine):
  DVE:  stage L[:, bi*128:+128] into the fixed Lstage tile (matmul weights
        APs must have static offsets -> vary the DATA, not the AP), then
        per gen one tensor_reduce(min) over the whole [128,4096] PSUM into
        MX/MY[:, bi] (register-offset slot).
  PE :  XX gen: 8 static 512-wide f32r matmuls psum[c] = Lstage.T @ RX[c],
        + eye matmul accumulating 2^30*I at the diag block (register
        offset out), then YX gen likewise from RY.

Sync: monotone counting semaphores with REGISTER-VALUED wait thresholds
(engine registers bumped per iteration). Decrement-based credit schemes
hang this runtime; register-valued waits are verified to work on it.

Host adds x2[i] (the augmented-matmul rows carry x2[j] split hi/lo so the
f32r operand rounding cannot corrupt it), clamps, and takes logs in fp64.
"""
import sys, time
sys.path.insert(0, '/opt/trn_rl_repo')

import numpy as np
from contextlib import ExitStack

import concourse.bass as bass
from concourse.bass import ds
from concourse import bacc, mybir
from concourse.bass_utils import run_bass_kernel_spmd

B, N, D = 8, 4096, 64
P = 128
NB = N // P             # 32 rounds
K = D + 2               # 66
NCH = N // 512          # 8 chunks per gen
EPS = 1e-12
SQRT_BIG = 32768.0      # 2^15; adds 2^30 on the XX diagonal

_cache = {}

f32 = mybir.dt.float32
f32r = mybir.dt.float32r


def _build(repeat=1, nb=NB):
    nc = bacc.Bacc(None, target_bir_lowering=False)
    L_d = nc.dram_tensor("L", [K, N], f32r, kind="ExternalInput")
    RX_d = nc.dram_tensor("RX", [K, N], f32r, kind="ExternalInput")
    RY_d = nc.dram_tensor("RY", [K, N], f32r, kind="ExternalInput")
    EYE_d = nc.dram_tensor("EYE", [P, P], f32r, kind="ExternalInput")
    MX_d = nc.dram_tensor("MX", [P, NB], f32, kind="ExternalOutput")
    MY_d = nc.dram_tensor("MY", [P, NB], f32, kind="ExternalOutput")

    n_rounds = nb * repeat
    n_gens = 2 * n_rounds

    with ExitStack() as ctx:
        Lr = ctx.enter_context(nc.sbuf_tensor([K, N], f32r))
        RXr = ctx.enter_context(nc.sbuf_tensor([K, N], f32r))
        RYr = ctx.enter_context(nc.sbuf_tensor([K, N], f32r))
        EYEr = ctx.enter_context(nc.sbuf_tensor([P, P], f32r))
        Lstage = ctx.enter_context(nc.sbuf_tensor([K, P], f32r))
        MX = ctx.enter_context(nc.sbuf_tensor([P, NB], f32))
        MY = ctx.enter_context(nc.sbuf_tensor([P, NB], f32))
        psum = ctx.enter_context(nc.psum_tensor([P, N], f32))
        dma_sem = ctx.enter_context(nc.semaphore())
        stage_done = ctx.enter_context(nc.semaphore())  # staged rounds
        pe_done = ctx.enter_context(nc.semaphore())     # completed PE gens
        dve_done = ctx.enter_context(nc.semaphore())    # completed reduces
        block = ctx.enter_context(nc.Block())

        @block.sync
        def _(sync):
            sync.dma_start(out=Lr[:], in_=L_d[:]).then_inc(dma_sem, 16)
            sync.dma_start(out=RXr[:], in_=RX_d[:]).then_inc(dma_sem, 16)
            sync.dma_start(out=RYr[:], in_=RY_d[:]).then_inc(dma_sem, 16)
            sync.dma_start(out=EYEr[:], in_=EYE_d[:]).then_inc(dma_sem, 16)
            sync.wait_ge(dve_done, n_gens)
            sync.dma_start(out=MX_d[:], in_=MX[:]).then_inc(dma_sem, 16)
            sync.dma_start(out=MY_d[:], in_=MY[:]).then_inc(dma_sem, 16)

        @block.vector
        def _(vector):
            vector.wait_ge(dma_sem, 64)
            ro = vector.alloc_register("ro")        # stage src offset: 128q
            svo = nc.snap(ro, donate=True, min_val=0, max_val=N - P)
            rsl = vector.alloc_register("rsl")      # mins slot: q
            svsl = nc.snap(rsl, donate=True, min_val=0, max_val=NB - 1)
            rsp = vector.alloc_register("rsp")      # pe gate stage: 2q
            svsp = nc.snap(rsp, donate=True, min_val=0, max_val=n_gens)
            rrx = vector.alloc_register("rrx")      # pe gate reduce XX: 2q+1
            svrx = nc.snap(rrx, donate=True, min_val=1, max_val=n_gens)
            rry = vector.alloc_register("rry")      # pe gate reduce YX: 2q+2
            svry = nc.snap(rry, donate=True, min_val=2, max_val=n_gens)
            vector.reg_mov(rsp, 0)
            vector.reg_mov(rrx, 1)
            vector.reg_mov(rry, 2)
            with vector.Fori(0, repeat):
                vector.reg_mov(ro, 0)
                vector.reg_mov(rsl, 0)
                with vector.Fori(0, nb):
                    # Lstage free once YX matmuls of round q-1 completed
                    vector.wait_ge(pe_done, svsp)
                    nc.vector.tensor_copy(
                        Lstage[:], Lr[:, ds(svo, P)]).then_inc(stage_done, 1)
                    vector.wait_ge(pe_done, svrx)
                    nc.vector.tensor_reduce(
                        out=MX[:, ds(svsl, 1)], in_=psum[:],
                        axis=mybir.AxisListType.X,
                        op=mybir.AluOpType.min).then_inc(dve_done, 1)
                    vector.wait_ge(pe_done, svry)
                    nc.vector.tensor_reduce(
                        out=MY[:, ds(svsl, 1)], in_=psum[:],
                        axis=mybir.AxisListType.X,
                        op=mybir.AluOpType.min).then_inc(dve_done, 1)
                    vector.reg_add(ro, ro, P)
                    vector.reg_add(rsl, rsl, 1)
                    vector.reg_add(rsp, rsp, 2)
                    vector.reg_add(rrx, rrx, 2)
                    vector.reg_add(rry, rry, 2)

        @block.tensor
        def _(tensor):
            tensor.wait_ge(dma_sem, 64)
            rd = tensor.alloc_register("rd")        # diag offset: 128q
            svd = nc.snap(rd, donate=True, min_val=0, max_val=N - P)
            rsd = tensor.alloc_register("rsd")      # stage gate: q+1
            svsd = nc.snap(rsd, donate=True, min_val=1, max_val=n_rounds)
            rxx = tensor.alloc_register("rxx")      # dve gate XX: 2q
            svxx = nc.snap(rxx, donate=True, min_val=0, max_val=n_gens)
            ryx = tensor.alloc_register("ryx")      # dve gate YX: 2q+1
            svyx = nc.snap(ryx, donate=True, min_val=1, max_val=n_gens)
            tensor.reg_mov(rsd, 1)
            tensor.reg_mov(rxx, 0)
            tensor.reg_mov(ryx, 1)
            with tensor.Fori(0, repeat):
                tensor.reg_mov(rd, 0)
                with tensor.Fori(0, nb):
                    # ---- XX gen ----
                    tensor.wait_ge(stage_done, svsd)
                    tensor.wait_ge(dve_done, svxx)
                    for c in range(NCH):
                        nc.tensor.matmul(
                            psum[:, c * 512:(c + 1) * 512], Lstage[:],
                            RXr[:, c * 512:(c + 1) * 512],
                            start=True, stop=True)
                    # diag mask: += 2^30 * I at cols [bi*128, +128)
                    nc.tensor.matmul(
                        psum[:, ds(svd, P)], EYEr[:], EYEr[:],
                        start=False, stop=True,
                        skip_group_check=True).then_inc(pe_done, 1)
                    # ---- YX gen ----
                    tensor.wait_ge(dve_done, svyx)
                    for c in range(NCH - 1):
                        nc.tensor.matmul(
                            psum[:, c * 512:(c + 1) * 512], Lstage[:],
                            RYr[:, c * 512:(c + 1) * 512],
                            start=True, stop=True)
                    c = NCH - 1
                    nc.tensor.matmul(
                        psum[:, c * 512:(c + 1) * 512], Lstage[:],
                        RYr[:, c * 512:(c + 1) * 512],
                        start=True, stop=True).then_inc(pe_done, 1)
                    tensor.reg_add(rd, rd, P)
                    tensor.reg_add(rsd, rsd, 1)
                    tensor.reg_add(rxx, rxx, 2)
                    tensor.reg_add(ryx, ryx, 2)

    nc.finalize()
    return nc


def _get_nc(repeat=1):
    if repeat not in _cache:
        _cache[repeat] = _build(repeat)
    return _cache[repeat]


def _prep_maps(X, Y):
    import ml_dtypes
    X = np.asarray(X, dtype=np.float32)
    Y = np.asarray(Y, dtype=np.float32)
    eye = (np.eye(P) * SQRT_BIG).astype(np.float32)
    in_maps = []
    x2_all = []
    for b in range(B):
        Xb = X[b].astype(np.float64)
        Yb = Y[b].astype(np.float64)
        x2 = (Xb * Xb).sum(1)
        y2 = (Yb * Yb).sum(1)
        ones = np.ones((1, N), dtype=np.float64)
        # hi part bf16-representable so it survives f32r operand rounding
        x2h = x2.astype(np.float32).astype(ml_dtypes.bfloat16).astype(np.float64)
        y2h = y2.astype(np.float32).astype(ml_dtypes.bfloat16).astype(np.float64)
        L = np.concatenate([-2.0 * Xb.T, ones, ones], 0).astype(np.float32)
        RX = np.concatenate([Xb.T, x2h[None], (x2 - x2h)[None]], 0).astype(np.float32)
        RY = np.concatenate([Yb.T, y2h[None], (y2 - y2h)[None]], 0).astype(np.float32)
        in_maps.append({"L": L, "RX": RX, "RY": RY, "EYE": eye})
        x2_all.append(x2)
    return in_maps, x2_all


def _postprocess(results, x2_all):
    out = np.zeros(B, dtype=np.float64)
    for b in range(B):
        mx = results[b]["MX"].astype(np.float64).T.reshape(-1)  # row i = bi*P+p
        my = results[b]["MY"].astype(np.float64).T.reshape(-1)
        x2 = x2_all[b]
        d2x = np.maximum(mx + x2, EPS)
        d2y = np.maximum(my + x2, EPS)
        out[b] = 0.5 * np.mean(np.log(d2x) - np.log(d2y))
    return out.astype(np.float32)


def _run_with_retry(nc, in_maps):
    last = None
    for attempt in range(3):
        try:
            return run_bass_kernel_spmd(nc, in_maps, core_ids=list(range(B))).results
        except Exception as ex:
            last = ex
            time.sleep(3)
    raise last


def kernel(X, Y):
    in_maps, x2_all = _prep_maps(X, Y)
    results = _run_with_retry(_get_nc(1), in_maps)
    return _postprocess(results, x2_all)


# Pre-build the default program at import time; guarded so import never fails.
try:
    _get_nc(1)
except Exception:
    pass


if __name__ == "__main__":
    rng = np.random.default_rng(0)
    X = rng.standard_normal((B, N, D)).astype(np.float32)
    Y = rng.standard_normal((B, N, D)).astype(np.float32)
    t0 = time.time()
    got = kernel(X, Y)
    print("kernel:", got, f"({time.time()-t0:.1f}s)")
    exp = np.zeros(B)
    for b in range(B):
        Xb, Yb = X[b].astype(np.float64), Y[b].astype(np.float64)
        x2 = (Xb**2).sum(1); y2 = (Yb**2).sum(1)
        vxx = x2[None] - 2*(Xb@Xb.T); np.fill_diagonal(vxx, 1e9)
        vyx = y2[None] - 2*(Xb@Yb.T)
        dx = np.maximum(vxx.min(1) + x2, EPS)
        dy = np.maximum(vyx.min(1) + x2, EPS)
        exp[b] = 0.5*np.mean(np.log(dx) - np.log(dy))
    print("exact ref:", exp)
    print("rel err:", np.linalg.norm(got-exp)/np.linalg.norm(exp))

